# revision 1
# baseline (speedup 1.0000x reference)
"""Trainium2 Bass kernel for nn_MultiHeadAttention (B=2, S=2048, D=1024, H=16).

Sharding: 8 cores = 2 batch groups x 4 cores. Core c handles batch c//4 and
heads 4*(c%4) .. 4*(c%4)+4. Each core computes Q/K/V projections for its
batch+heads, transposed-layout attention (softmax denominators via a
ones-augmented V column), and a partial output projection over its 256
head-dims. Host sums the 4 partials per batch (tensor-parallel unshard).

All matmuls run in float32r (1 cycle/row on the PE at N>=512; ~11-bit
mantissa inputs, fp32 accumulation).
"""

import numpy as np

B, S, D, H = 2, 2048, 1024, 16
HD = D // H          # 64
NCORES = 8
HPC = 4              # heads per core
CHD = HPC * HD       # 256 head-dims per core
TOK = S              # tokens per core (one batch)
QW = 512             # query window
NQW = TOK // QW      # 4 windows
NKT = TOK // 128     # 16 key tiles
SCALE = 1.0 / np.sqrt(np.float32(D))  # 1/32

_PROG = None  # cached compiled program
_LAST_IN_MAPS = None  # stashed per-core inputs (for external profiling)


def _round_fp32r(x: np.ndarray) -> np.ndarray:
    """Round fp32 to fp32r (11-bit mantissa, RNE) so matmul inputs match the
    PE's fp32r datapath."""
    x = np.ascontiguousarray(x, dtype=np.float32)
    u = x.view(np.uint32)
    r = (u + np.uint32(0x800) + ((u >> np.uint32(12)) & np.uint32(1))) & np.uint32(
        0xFFFFF000
    )
    return r.view(np.float32)


def _build():
    from contextlib import ExitStack

    import concourse.bass as bass
    import concourse.tile as tile
    from concourse import bacc, mybir

    F32R = mybir.dt.float32r
    F32 = mybir.dt.float32
    EXP = mybir.ActivationFunctionType.Exp

    nc = bacc.Bacc("TRN2", target_bir_lowering=False, debug=False,
                   num_devices=NCORES)

    xqT = nc.dram_tensor("xqT", [D, TOK], F32R, kind="ExternalInput").ap()
    xkT = nc.dram_tensor("xkT", [D, TOK], F32R, kind="ExternalInput").ap()
    xvT = nc.dram_tensor("xvT", [D, TOK], F32R, kind="ExternalInput").ap()
    wqT = nc.dram_tensor("wqT", [D, CHD], F32R, kind="ExternalInput").ap()
    wkT = nc.dram_tensor("wkT", [D, CHD], F32R, kind="ExternalInput").ap()
    wvT = nc.dram_tensor("wvT", [D, CHD], F32R, kind="ExternalInput").ap()
    woTs = nc.dram_tensor("woTs", [CHD, D], F32R, kind="ExternalInput").ap()
    id128 = nc.dram_tensor("id128", [128, 128], F32R, kind="ExternalInput").ap()
    pout = nc.dram_tensor("pout", [TOK, D], F32, kind="ExternalOutput").ap()

    CB = 256            # projection column-block width
    NCB = TOK // CB     # 8 blocks

    with tile.TileContext(nc) as tc, ExitStack() as ctx:
        const = ctx.enter_context(tc.tile_pool(name="const", bufs=1))
        wq_sb = const.tile([128, 8, CHD], F32R, tag="wq")
        wk_sb = const.tile([128, 8, CHD], F32R, tag="wk")
        wv_sb = const.tile([128, 8, CHD], F32R, tag="wv")
        nc.sync.dma_start(out=wk_sb, in_=wkT.rearrange("(ko ki) m -> ki ko m", ki=128))
        nc.sync.dma_start(out=wv_sb, in_=wvT.rearrange("(ko ki) m -> ki ko m", ki=128))
        nc.sync.dma_start(out=wq_sb, in_=wqT.rearrange("(ko ki) m -> ki ko m", ki=128))
        wo_sb = [const.tile([128, D], F32R, tag=f"wo{p}", name=f"wo{p}")
                 for p in range(2)]
        for p in range(2):
            nc.sync.dma_start(out=wo_sb[p], in_=woTs[p * 128:(p + 1) * 128, :])

        # warm the exp table early
        warm = const.tile([1, 8], F32)
        nc.vector.memset(warm, 0.0)
        nc.scalar.activation(out=warm, in_=warm, func=EXP)

        kqt = ctx.enter_context(tc.tile_pool(name="kqt", bufs=1))
        KT = [kqt.tile([128, TOK], F32R, tag=f"kt{m}", name=f"KT{m}") for m in range(2)]
        vnat = kqt.tile([128, NKT, 4 * (HD + 1)], F32R, tag="vnat")
        ctxP = [kqt.tile([128, TOK], F32R, tag=f"ctxP{p}", name=f"ctxP{p}")
                for p in range(2)]
        ones16 = const.tile([128, NKT], F32)
        nc.vector.memset(ones16, 1.0)
        for h in range(HPC):
            nc.vector.tensor_copy(vnat[:, :, h * 65 + 64], ones16[:])

        # shared small-psum pool: projections, Q windows, out-projection
        pp = ctx.enter_context(tc.tile_pool(name="pp", bufs=2, space="PSUM"))
        xblk = ctx.enter_context(tc.tile_pool(name="xblk", bufs=3))
        qtw = ctx.enter_context(tc.tile_pool(name="qtw", bufs=4))
        qx = ctx.enter_context(tc.tile_pool(name="qx", bufs=2))

        def project_q_window(qw):
            q0 = qw * QW
            qxb = qx.tile([128, 8, QW], F32R, tag="qx", name=f"qx{qw}")
            nc.sync.dma_start(
                out=qxb,
                in_=xqT[:, q0:q0 + QW].rearrange("(ko ki) t -> ki ko t", ki=128))
            qt_win = []
            for m in range(2):
                qp = pp.tile([128, QW], F32, tag="pp", name=f"qp{qw}_{m}")
                for ko in range(8):
                    nc.tensor.matmul(
                        qp[:], wq_sb[:, ko, m * 128:(m + 1) * 128],
                        qxb[:, ko, :], start=(ko == 0), stop=(ko == 7))
                qt = qtw.tile([128, QW], F32R, tag="qt", name=f"qt{qw}_{m}")
                nc.vector.tensor_copy(qt[:], qp[:])
                qt_win.append(qt)
            return qt_win

        # Q for window 0 first (tiny DMA) so attention can start with block 0
        qt0_win = project_q_window(0)

        # ---- interleaved K/V column blocks ----
        for blk in range(NCB):
            c0 = blk * CB
            # K block: K.T[:, c0:c0+CB] for both head-pair tiles
            xbk = xblk.tile([128, 8, CB], F32R, tag="xb", name=f"xbk{blk}")
            nc.sync.dma_start(
                out=xbk,
                in_=xkT[:, c0:c0 + CB].rearrange("(ko ki) t -> ki ko t", ki=128))
            psk = pp.tile([128, 2 * CB], F32, tag="pp", name=f"psk{blk}")
            for m in range(2):
                for ko in range(8):
                    nc.tensor.matmul(
                        psk[:, m * CB:(m + 1) * CB],
                        wk_sb[:, ko, m * 128:(m + 1) * 128],
                        xbk[:, ko, :], start=(ko == 0), stop=(ko == 7))
            for m in range(2):
                nc.vector.tensor_copy(
                    KT[m][:, c0:c0 + CB], psk[:, m * CB:(m + 1) * CB])
            # V block: natural-layout V via x.T-stationary matmuls
            xbv = xblk.tile([128, 8, CB], F32R, tag="xb", name=f"xbv{blk}")
            nc.sync.dma_start(
                out=xbv,
                in_=xvT[:, c0:c0 + CB].rearrange("(ko ki) t -> ki ko t", ki=128))
            for tt in range(2):
                bi = 2 * blk + tt
                pv = pp.tile([128, CHD], F32, tag="pp", name=f"pv{bi}")
                for ko in range(8):
                    nc.tensor.matmul(
                        pv[:], xbv[:, ko, tt * 128:(tt + 1) * 128],
                        wv_sb[:, ko, :], start=(ko == 0), stop=(ko == 7))
                for h in range(HPC):
                    nc.vector.tensor_copy(
                        vnat[:, bi, h * 65:h * 65 + 64],
                        pv[:, h * HD:(h + 1) * HD])

        # ---- attention windows ----
        with tc.tile_pool(name="sc_ps", bufs=2, space="PSUM") as sc_ps, \
             tc.tile_pool(name="pt", bufs=4) as ptp, \
             tc.tile_pool(name="cop", bufs=2, space="PSUM") as cop, \
             tc.tile_pool(name="nrm", bufs=2) as nrm, \
             tc.tile_pool(name="rdp", bufs=2, space="DRAM") as rdp, \
             tc.tile_pool(name="bc", bufs=4) as bcp, \
             tc.tile_pool(name="scr", bufs=2) as scrp, \
             tc.tile_pool(name="oev", bufs=3) as oev:

            def emit_outproj(qw):
                q0 = qw * QW
                for tt in range(QW // 128):
                    t0 = q0 + tt * 128
                    for et in range(2):
                        po = pp.tile([128, 512], F32, tag="pp",
                                     name=f"po{qw}_{tt}_{et}")
                        for p in range(2):
                            nc.tensor.matmul(
                                po[:],
                                ctxP[p][:, t0:t0 + 128],
                                wo_sb[p][:, et * 512:(et + 1) * 512],
                                start=(p == 0), stop=(p == 1))
                        ot = oev.tile([128, 512], F32, tag="ot")
                        nc.vector.tensor_copy(ot[:], po[:])
                        nc.sync.dma_start(
                            out=pout[t0:t0 + 128, et * 512:(et + 1) * 512],
                            in_=ot[:])

            for qw in range(NQW):
                q0 = qw * QW
                qt_win = qt0_win if qw == 0 else project_q_window(qw)

                for p in range(2):
                    hA, hB = 2 * p, 2 * p + 1
                    cA = cop.tile([65, QW], F32, tag="ctx")
                    cB = cop.tile([65, QW], F32, tag="ctx")
                    for kt in range(NKT):
                        sc = sc_ps.tile([128, 2 * QW], F32, tag="sc")
                        nc.tensor.matmul(
                            sc[:, 0:QW],
                            KT[p][0:64, kt * 128:(kt + 1) * 128],
                            qt_win[p][0:64, :],
                            start=True, stop=True, tile_position=(0, 0))
                        nc.tensor.matmul(
                            sc[:, QW:2 * QW],
                            KT[p][64:128, kt * 128:(kt + 1) * 128],
                            qt_win[p][64:128, :],
                            start=True, stop=True, tile_position=(64, 0))
                        pt = ptp.tile([128, 2 * QW], F32R, tag="pt")
                        nc.scalar.activation(out=pt[:], in_=sc[:], func=EXP,
                                             scale=float(SCALE))
                        nc.tensor.matmul(
                            cA[:], vnat[:, kt, hA * 65:hA * 65 + 65],
                            pt[:, 0:QW], start=(kt == 0), stop=(kt == NKT - 1))
                        nc.tensor.matmul(
                            cB[:], vnat[:, kt, hB * 65:hB * 65 + 65],
                            pt[:, QW:2 * QW], start=(kt == 0),
                            stop=(kt == NKT - 1))
                    # evict ctx+den, then normalize from SBUF
                    eA = nrm.tile([65, QW], F32, tag="eA")
                    eB = nrm.tile([65, QW], F32, tag="eB")
                    nc.vector.tensor_copy(eA[:], cA[:])
                    nc.vector.tensor_copy(eB[:], cB[:])
                    rstA = nrm.tile([1, QW], F32, tag="rstA")
                    rstB = nrm.tile([1, QW], F32, tag="rstB")
                    nc.vector.tensor_copy(rstA[:], eA[64:65, :])
                    nc.vector.tensor_copy(rstB[:], eB[64:65, :])
                    rrcA = nrm.tile([1, QW], F32, tag="rrcA")
                    rrcB = nrm.tile([1, QW], F32, tag="rrcB")
                    nc.vector.reciprocal_approx_fast(rrcA[:], rstA[:])
                    nc.vector.reciprocal_approx_fast(rrcB[:], rstB[:])
                    rden = rdp.tile([2, QW], F32, tag="rden")
                    nc.sync.dma_start(out=rden[0:1, :], in_=rrcA[:])
                    nc.sync.dma_start(out=rden[1:2, :], in_=rrcB[:])
                    bcA = bcp.tile([64, QW], F32, tag="bc")
                    bcB = bcp.tile([64, QW], F32, tag="bc")
                    nc.sync.dma_start(
                        out=bcA,
                        in_=bass.AP(tensor=rden.tensor, offset=rden.offset,
                                    ap=[[0, 64], [1, QW]]))
                    nc.sync.dma_start(
                        out=bcB,
                        in_=bass.AP(tensor=rden.tensor,
                                    offset=rden.offset + QW,
                                    ap=[[0, 64], [1, QW]]))
                    nc.vector.tensor_mul(
                        ctxP[p][0:64, q0:q0 + QW], eA[0:64, :], bcA[:])
                    scb = scrp.tile([64, QW], F32R, tag="scb")
                    nc.vector.tensor_mul(scb[:], eB[0:64, :], bcB[:])
                    nc.sync.dma_start(
                        out=ctxP[p][64:128, q0:q0 + QW], in_=scb[:])
                    if p == 0 and qw > 0:
                        emit_outproj(qw - 1)
            emit_outproj(NQW - 1)

    nc.compile()
    return nc


def kernel(query, key, value, Wq, Wk, Wv, Wo):
    global _PROG
    from concourse.bass_utils import run_bass_kernel_spmd

    if _PROG is None:
        _PROG = _build()
    nc = _PROG

    q2 = np.asarray(query, dtype=np.float32).reshape(B, S, D)
    k2 = np.asarray(key, dtype=np.float32).reshape(B, S, D)
    v2 = np.asarray(value, dtype=np.float32).reshape(B, S, D)
    Wq = np.asarray(Wq, dtype=np.float32)
    Wk = np.asarray(Wk, dtype=np.float32)
    Wv = np.asarray(Wv, dtype=np.float32)
    Wo = np.asarray(Wo, dtype=np.float32)
    ident = _round_fp32r(np.eye(128, dtype=np.float32))

    xT = {}
    for b in range(B):
        xT[("q", b)] = _round_fp32r(q2[b].T)
        xT[("k", b)] = _round_fp32r(k2[b].T)
        xT[("v", b)] = _round_fp32r(v2[b].T)

    in_maps = []
    for c in range(NCORES):
        b = c // 4
        l = c % 4
        rs = slice(CHD * l, CHD * (l + 1))
        in_maps.append({
            "xqT": xT[("q", b)],
            "xkT": xT[("k", b)],
            "xvT": xT[("v", b)],
            "wqT": _round_fp32r(Wq[rs, :].T),
            "wkT": _round_fp32r(Wk[rs, :].T),
            "wvT": _round_fp32r(Wv[rs, :].T),
            "woTs": _round_fp32r(Wo[:, rs].T),
            "id128": ident,
        })

    global _LAST_IN_MAPS
    _LAST_IN_MAPS = in_maps
    res = run_bass_kernel_spmd(nc, in_maps, core_ids=list(range(NCORES)))
    parts = [res.results[c]["pout"] for c in range(NCORES)]
    out = np.empty((B, S, D), dtype=np.float32)
    for b in range(B):
        out[b] = parts[4 * b] + parts[4 * b + 1] + parts[4 * b + 2] + parts[4 * b + 3]
    return out



# revision 14
# speedup vs baseline: 1.1899x; 1.1899x over previous
"""Trainium2 Bass kernel for nn_MultiHeadAttention (B=2, S=2048, D=1024, H=16).

Sharding: 8 cores = 2 batch groups x 4 cores. Core c handles batch c//4 and
heads 4*(c%4) .. 4*(c%4)+4 (two head-pairs p=0,1). Each core computes Q/K/V
projections for its batch+heads, transposed-layout attention, and a partial
output projection over its 256 head-dims. Host sums the 4 partials per batch.

v2 design (vs the f32r baseline):
- fp16 inputs/weights (halves HBM traffic + SBUF), softmax scale folded into Wq.
- exp runs on ACT directly PSUM->SBUF as fp16 pt tiles (no separate evict).
- attn@V matmuls are column-tiled pairs (head A -> PE cols 0:64, head B ->
  cols 64:128) running concurrently at full array width (M=64 each, one
  [128,512] PSUM tile holds both heads' context).
- softmax denominators: DVE pairwise-tree over the 16 fp16 pt tiles, then a
  ones[128,2] matmul for the 128-key partition reduction, reciprocal on DVE,
  and a selector-matmul broadcast; one DVE multiply normalizes both heads
  (head B lands on partitions 64:128 -> no DMA broadcast round-trip).
- out-projection partials are DMA'd directly PSUM->HBM.
- K-projection+Q0 ramp interleaved with the first attention block's scores;
  V/Q projections and out-projection woven into PE slack of the ACT-paced
  attention sweep.
"""

import numpy as np

B, S, D, H = 2, 2048, 1024, 16
HD = D // H          # 64
NCORES = 8
HPC = 4              # heads per core
CHD = HPC * HD       # 256 head-dims per core
TOK = S              # tokens per core (one batch)
QW = 512             # query window
NQW = TOK // QW      # 4 windows
NKT = TOK // 128     # 16 key tiles
SCALE = 1.0 / np.sqrt(np.float32(D))  # 1/32, folded into Wq on host

_PROG = None  # cached compiled program
_LAST_IN_MAPS = None  # stashed per-core inputs (for external profiling)


def _build():
    from contextlib import ExitStack

    import concourse.bass as bass
    import concourse.tile as tile
    from concourse import bacc, mybir

    F16 = mybir.dt.float16
    F32 = mybir.dt.float32
    EXP = mybir.ActivationFunctionType.Exp

    nc = bacc.Bacc("TRN2", target_bir_lowering=False, debug=False,
                   num_devices=NCORES)

    xqT = nc.dram_tensor("xqT", [D, TOK], F16, kind="ExternalInput").ap()
    xkT = nc.dram_tensor("xkT", [D, TOK], F16, kind="ExternalInput").ap()
    xvT = nc.dram_tensor("xvT", [D, TOK], F16, kind="ExternalInput").ap()
    wqT = nc.dram_tensor("wqT", [D, CHD], F16, kind="ExternalInput").ap()
    wkT = nc.dram_tensor("wkT", [D, CHD], F16, kind="ExternalInput").ap()
    wvT = nc.dram_tensor("wvT", [D, CHD], F16, kind="ExternalInput").ap()
    woTs = nc.dram_tensor("woTs", [CHD, D], F16, kind="ExternalInput").ap()
    pout = nc.dram_tensor("pout", [TOK, D], F16, kind="ExternalOutput").ap()

    CB = 256            # projection column-block width (tokens)
    NCB = TOK // CB     # 8 blocks

    with tile.TileContext(nc) as tc, ExitStack() as ctx:
        const = ctx.enter_context(tc.tile_pool(name="const", bufs=1))
        wq_sb = const.tile([128, 8, CHD], F16, tag="wq")
        wk_sb = const.tile([128, 8, CHD], F16, tag="wk")
        wv_sb = const.tile([128, 8, CHD], F16, tag="wv")
        wo_sb = [const.tile([128, D], F16, tag=f"wo{p}", name=f"wo{p}")
                 for p in range(2)]
        # ones for the denominator partition-sum matmul
        onesK = const.tile([128, 1], F16, tag="onesK")
        # ones row for the reciprocal broadcast matmuls
        ones1 = const.tile([1, 128], F32, tag="ones1")

        nc.sync.dma_start(out=wk_sb, in_=wkT.rearrange("(ko ki) m -> ki ko m", ki=128))
        nc.sync.dma_start(out=wq_sb, in_=wqT.rearrange("(ko ki) m -> ki ko m", ki=128))
        nc.sync.dma_start(out=wv_sb, in_=wvT.rearrange("(ko ki) m -> ki ko m", ki=128))
        for p in range(2):
            nc.sync.dma_start(out=wo_sb[p], in_=woTs[p * 128:(p + 1) * 128, :])

        nc.vector.memset(onesK, 1.0)
        nc.vector.memset(ones1, 1.0)

        # warm the exp table early
        warm = const.tile([1, 8], F32, tag="warm")
        nc.vector.memset(warm, 0.0)
        nc.scalar.activation(out=warm, in_=warm, func=EXP)

        big = ctx.enter_context(tc.tile_pool(name="big", bufs=1))
        KT = big.tile([128, 2, TOK], F16, tag="kt")          # [hd, pair, keys]
        vnat = big.tile([128, NKT, CHD], F16, tag="vnat")    # [keys, kt, hd]
        ctxP = [big.tile([128, TOK], F16, tag=f"ctxP{p}", name=f"ctxP{p}")
                for p in range(2)]

        xkb = ctx.enter_context(tc.tile_pool(name="xkb", bufs=8))
        xvb = ctx.enter_context(tc.tile_pool(name="xvb", bufs=4))
        xqb = ctx.enter_context(tc.tile_pool(name="xqb", bufs=2))
        qtp = ctx.enter_context(tc.tile_pool(name="qtp", bufs=2))
        ptp = ctx.enter_context(tc.tile_pool(name="ptp", bufs=2))
        dtp = ctx.enter_context(tc.tile_pool(name="dtp", bufs=2))
        rrp = ctx.enter_context(tc.tile_pool(name="rrp", bufs=2))
        oev = ctx.enter_context(tc.tile_pool(name="oev", bufs=4))

        # PSUM (8 banks): sc ring 2x[128,1024] (4) + cop ring 2x[128,512] (2)
        # + shared proj/out/den/bcast pool 2x[128,512] (2)
        scp = ctx.enter_context(tc.tile_pool(name="scp", bufs=2, space="PSUM"))
        copp = ctx.enter_context(tc.tile_pool(name="copp", bufs=2, space="PSUM"))
        pp = ctx.enter_context(tc.tile_pool(name="pp", bufs=2, space="PSUM"))

        # ---------- emission helpers ----------
        def dma_x_block(pool, src, c0, w, name):
            t = pool.tile([128, 8, w], F16, tag="xb", name=name)
            nc.sync.dma_start(
                out=t,
                in_=src[:, c0:c0 + w].rearrange("(ko ki) t -> ki ko t", ki=128))
            return t

        def proj_q(qw_i, xq_t):
            """Project one query window -> qt [128, 2, QW] fp16."""
            qt = qtp.tile([128, 2, QW], F16, tag="qt", name=f"qt{qw_i}")
            for m in range(2):
                psq = pp.tile([128, QW], F32, tag="pp", name=f"psq{qw_i}_{m}")
                for ko in range(8):
                    nc.tensor.matmul(
                        psq[:], wq_sb[:, ko, m * 128:(m + 1) * 128],
                        xq_t[:, ko, :], start=(ko == 0), stop=(ko == 7))
                nc.vector.tensor_copy(qt[:, m, :], psq[:])
            return qt

        def proj_k_block(blk, xk_t):
            c0 = blk * CB
            psk = pp.tile([128, 2 * CB], F32, tag="pp", name=f"psk{blk}")
            for m in range(2):
                for ko in range(8):
                    nc.tensor.matmul(
                        psk[:, m * CB:(m + 1) * CB],
                        wk_sb[:, ko, m * 128:(m + 1) * 128],
                        xk_t[:, ko, :], start=(ko == 0), stop=(ko == 7))
            nc.vector.tensor_copy(
                KT[:, :, c0:c0 + CB],
                bass.AP(tensor=psk.tensor, offset=psk.offset,
                        ap=[list(psk.ap[0]), [CB, 2], [1, CB]]))

        def proj_v_block(blk, xv_t):
            # out = x_blockT.T @ Wv -> [256 tok, 256 hd] natural layout
            pv = pp.tile([128, 2 * CHD], F32, tag="pp", name=f"pv{blk}")
            for tt in range(2):
                for ko in range(8):
                    nc.tensor.matmul(
                        pv[:, tt * CHD:(tt + 1) * CHD],
                        xv_t[:, ko, tt * 128:(tt + 1) * 128],
                        wv_sb[:, ko, :], start=(ko == 0), stop=(ko == 7))
            nc.vector.tensor_copy(
                vnat[:, 2 * blk:2 * blk + 2, :],
                bass.AP(tensor=pv.tensor, offset=pv.offset,
                        ap=[list(pv.ap[0]), [CHD, 2], [1, CHD]]))

        def emit_scores(p, kt_i, qt, name):
            k0 = kt_i * 128
            sc = scp.tile([128, 2 * QW], F32, tag="sc", name=name)
            nc.tensor.matmul(
                sc[:, 0:QW], KT[0:64, p, k0:k0 + 128], qt[0:64, p, :],
                start=True, stop=True, tile_position=(0, 0))
            nc.tensor.matmul(
                sc[:, QW:2 * QW], KT[64:128, p, k0:k0 + 128], qt[64:128, p, :],
                start=True, stop=True, tile_position=(64, 0))
            return sc

        def emit_exp(sc, ptblk, kt_i):
            nc.scalar.activation(out=ptblk[:, kt_i, :], in_=sc[:], func=EXP)

        def emit_ctx(p, kt_i, ptblk, cop):
            h0 = p * 128
            nc.tensor.matmul(
                cop[0:64, :], vnat[:, kt_i, h0:h0 + 64],
                ptblk[:, kt_i, 0:QW],
                start=(kt_i == 0), stop=(kt_i == NKT - 1),
                tile_position=(0, 0))
            nc.tensor.matmul(
                cop[64:128, :], vnat[:, kt_i, h0 + 64:h0 + 128],
                ptblk[:, kt_i, QW:2 * QW],
                start=(kt_i == 0), stop=(kt_i == NKT - 1),
                tile_position=(0, 64))

        def emit_den_norm(p, qw_i, ptblk, cop, blk_i):
            """DVE tree-sum of pt over kt, partition-reduce via ones-matmul,
            reciprocal, selector-broadcast matmul, normalize into ctxP."""
            st = dtp.tile([128, 4, 2 * QW], F16, tag="dt", name=f"dt{blk_i}")
            pv = lambda a, b: ptblk[:, a:b, :]
            # stage 1: 4 adds, each [128, 2, 1024]
            nc.vector.tensor_add(st[:, 0:2, :], pv(0, 2), pv(2, 4))
            nc.vector.tensor_add(st[:, 2:4, :], pv(4, 6), pv(6, 8))
            nc.vector.tensor_add(pv(0, 2), pv(8, 10), pv(10, 12))
            nc.vector.tensor_add(pv(2, 4), pv(12, 14), pv(14, 16))
            # stage 2
            nc.vector.tensor_add(st[:, 0:2, :], st[:, 0:2, :], st[:, 2:4, :])
            nc.vector.tensor_add(pv(0, 2), pv(0, 2), pv(2, 4))
            # stage 3
            nc.vector.tensor_add(st[:, 0:2, :], st[:, 0:2, :], pv(0, 2))
            # stage 4: acc [128, 1024] = st[:,0,:] + st[:,1,:]
            nc.vector.tensor_add(st[:, 2, :], st[:, 0, :], st[:, 1, :])
            accv = st[:, 2, :]
            # partition reduce: den_h [1, 512] = ones.T @ acc_h, per head
            # (everything stays on partition 0: engines can't address a
            # partition range starting at an unaligned base)
            rrc = rrp.tile([1, 2 * QW], F32, tag="rrc", name=f"rrc{blk_i}")
            for h in range(2):
                den = pp.tile([128, QW], F32, tag="pp", name=f"den{blk_i}_{h}")
                nc.tensor.matmul(
                    den[0:1, :], onesK[:, 0:1], accv[:, h * QW:(h + 1) * QW],
                    start=True, stop=True)
                nc.vector.reciprocal_approx_fast(
                    rrc[0:1, h * QW:(h + 1) * QW], den[0:1, :])
            # broadcast via two ones-matmuls: head A -> parts 0:64, B -> 64:128
            bc = pp.tile([128, QW], F32, tag="pp", name=f"bc{blk_i}")
            nc.tensor.matmul(bc[0:64, :], ones1[0:1, 0:64], rrc[0:1, 0:QW],
                             start=True, stop=True, tile_position=(0, 0))
            nc.tensor.matmul(bc[64:128, :], ones1[0:1, 0:64],
                             rrc[0:1, QW:2 * QW],
                             start=True, stop=True, tile_position=(0, 64))
            # both tensor_tensor operands can't be PSUM -> stage bc in SBUF
            bcs = rrp.tile([128, QW], F32, tag="bcs", name=f"bcs{blk_i}")
            nc.vector.tensor_copy(bcs[:], bc[:])
            # normalize both heads at once into ctxP (fp16)
            nc.vector.tensor_mul(
                ctxP[p][:, qw_i * QW:(qw_i + 1) * QW], cop[:], bcs[:])

        def emit_outproj_one(qw_i, c):
            # c in 0..7 enumerates (tt, et)
            tt, et = c // 2, c % 2
            t0 = qw_i * QW + tt * 128
            po = pp.tile([128, 512], F32, tag="pp",
                         name=f"po{qw_i}_{tt}_{et}")
            for p in range(2):
                nc.tensor.matmul(
                    po[:], ctxP[p][:, t0:t0 + 128],
                    wo_sb[p][:, et * 512:(et + 1) * 512],
                    start=(p == 0), stop=(p == 1))
            # gpsimd has no PSUM port -> evict on DVE (fp16), DMA from SBUF
            ev = oev.tile([128, 512], F16, tag="oev")
            nc.vector.tensor_copy(ev[:], po[:])
            nc.sync.dma_start(
                out=pout[t0:t0 + 128, et * 512:(et + 1) * 512],
                in_=ev[:])

        def emit_outproj(qw_i):
            for c in range(8):
                emit_outproj_one(qw_i, c)

        # ---------- program ----------
        # DMA order: Q window 0 (feeds first scores), all K blocks, V blocks,
        # later Q windows (woven in during attention).
        xq_t = [None] * NQW
        xq_t[0] = dma_x_block(xqb, xqT, 0, QW, "xq0")
        xk_t = [dma_x_block(xkb, xkT, b * CB, CB, f"xk{b}") for b in range(NCB)]

        qt0 = proj_q(0, xq_t[0])

        # ramp: K projection interleaved with block 0's scores+exp so ACT
        # starts as early as possible
        ptblk0 = ptp.tile([128, NKT, 2 * QW], F16, tag="pt", name="pt0")
        for b in range(NCB):
            proj_k_block(b, xk_t[b])
            sc = emit_scores(0, 2 * b, qt0, f"sc_r{b}a")
            emit_exp(sc, ptblk0, 2 * b)
            sc = emit_scores(0, 2 * b + 1, qt0, f"sc_r{b}b")
            emit_exp(sc, ptblk0, 2 * b + 1)

        xv_t = [dma_x_block(xvb, xvT, b * CB, CB, f"xv{b}") for b in range(4)]
        xq_t[1] = dma_x_block(xqb, xqT, QW, QW, "xq1")

        # V projection + block 0's ctx (trails the exps produced above)
        cop0 = copp.tile([128, QW], F32, tag="cop", name="cop0")
        for b in range(NCB):
            if b >= 4:
                xv_t.append(dma_x_block(xvb, xvT, b * CB, CB, f"xv{b}"))
            proj_v_block(b, xv_t[b])
            emit_ctx(0, 2 * b, ptblk0, cop0)
            emit_ctx(0, 2 * b + 1, ptblk0, cop0)

        qt_cur = qt0
        qt_next = None
        # pending work from the previous block
        pend_norm = (0, 0, ptblk0, cop0)   # (p, qw, ptblk, cop)
        pend_ctx = None                    # (p, ptblk, cop): last kt pair

        blocks = [(qw_i, p) for qw_i in range(NQW) for p in range(2)][1:]
        for bi, (qw_i, p) in enumerate(blocks):
            blk_i = bi + 1
            ptblk = ptp.tile([128, NKT, 2 * QW], F16, tag="pt",
                             name=f"pt{blk_i}")
            cop = copp.tile([128, QW], F32, tag="cop", name=f"cop{blk_i}")
            qt_b = qt_cur
            for j in range(NKT // 2):
                sc = emit_scores(p, 2 * j, qt_b, f"sc{blk_i}_{j}a")
                emit_exp(sc, ptblk, 2 * j)
                sc = emit_scores(p, 2 * j + 1, qt_b, f"sc{blk_i}_{j}b")
                emit_exp(sc, ptblk, 2 * j + 1)
                if j == 0:
                    if pend_ctx is not None:
                        # previous block's last kt pair (after this block's
                        # first scores so ACT never idles at the boundary)
                        emit_ctx(pend_ctx[0], NKT - 2, pend_ctx[1], pend_ctx[2])
                        emit_ctx(pend_ctx[0], NKT - 1, pend_ctx[1], pend_ctx[2])
                        pend_ctx = None
                else:
                    emit_ctx(p, 2 * j - 2, ptblk, cop)
                    emit_ctx(p, 2 * j - 1, ptblk, cop)
                if j == 2 and pend_norm is not None:
                    emit_den_norm(pend_norm[0], pend_norm[1], pend_norm[2],
                                  pend_norm[3], blk_i - 1)
                    pend_norm = None
                if p == 1 and qw_i >= 1 and 3 <= j <= 6:
                    # previous window's out-projection, spread over 4 groups
                    # so the PSUM-pool DMA drains never stall the PE long
                    emit_outproj_one(qw_i - 1, 2 * (j - 3))
                    emit_outproj_one(qw_i - 1, 2 * (j - 3) + 1)
                if j == 5 and p == 0 and qw_i + 1 < NQW:
                    xq_t[qw_i + 1] = dma_x_block(
                        xqb, xqT, (qw_i + 1) * QW, QW, f"xq{qw_i + 1}")
                if j == 6 and p == 1 and qw_i + 1 < NQW:
                    qt_next = proj_q(qw_i + 1, xq_t[qw_i + 1])
            pend_ctx = (p, ptblk, cop)
            pend_norm = (p, qw_i, ptblk, cop)
            if p == 1 and qw_i + 1 < NQW:
                qt_cur = qt_next

        # tail: last block's final ctx pair, den/norm, last window out-proj
        emit_ctx(pend_ctx[0], NKT - 2, pend_ctx[1], pend_ctx[2])
        emit_ctx(pend_ctx[0], NKT - 1, pend_ctx[1], pend_ctx[2])
        emit_den_norm(pend_norm[0], pend_norm[1], pend_norm[2],
                      pend_norm[3], 8)
        emit_outproj(NQW - 1)

    nc.compile()
    return nc


def kernel(query, key, value, Wq, Wk, Wv, Wo):
    global _PROG, _LAST_IN_MAPS
    from concourse.bass_utils import run_bass_kernel_spmd

    if _PROG is None:
        _PROG = _build()
    nc = _PROG

    q2 = np.asarray(query, dtype=np.float32).reshape(B, S, D)
    k2 = np.asarray(key, dtype=np.float32).reshape(B, S, D)
    v2 = np.asarray(value, dtype=np.float32).reshape(B, S, D)
    Wq = np.asarray(Wq, dtype=np.float32)
    Wk = np.asarray(Wk, dtype=np.float32)
    Wv = np.asarray(Wv, dtype=np.float32)
    Wo = np.asarray(Wo, dtype=np.float32)

    xT = {}
    for b in range(B):
        xT[("q", b)] = np.ascontiguousarray(q2[b].T).astype(np.float16)
        xT[("k", b)] = np.ascontiguousarray(k2[b].T).astype(np.float16)
        xT[("v", b)] = np.ascontiguousarray(v2[b].T).astype(np.float16)

    in_maps = []
    for c in range(NCORES):
        b = c // 4
        l = c % 4
        rs = slice(CHD * l, CHD * (l + 1))
        in_maps.append({
            "xqT": xT[("q", b)],
            "xkT": xT[("k", b)],
            "xvT": xT[("v", b)],
            "wqT": (Wq[rs, :].T * SCALE).astype(np.float16),
            "wkT": Wk[rs, :].T.astype(np.float16),
            "wvT": Wv[rs, :].T.astype(np.float16),
            "woTs": np.ascontiguousarray(Wo[:, rs].T).astype(np.float16),
        })

    _LAST_IN_MAPS = in_maps
    res = run_bass_kernel_spmd(nc, in_maps, core_ids=list(range(NCORES)))
    parts = [res.results[c]["pout"].astype(np.float32) for c in range(NCORES)]
    out = np.empty((B, S, D), dtype=np.float32)
    for b in range(B):
        out[b] = parts[4 * b] + parts[4 * b + 1] + parts[4 * b + 2] + parts[4 * b + 3]
    return out
